# revision 9
# baseline (speedup 1.0000x reference)
"""Trainium2 Bass kernel for nn_EngramMemory (n-gram hash embedding + gated memory).

Contract: kernel(**inputs) takes FULL unsharded inputs (as produced by
setup_inputs) and returns the FULL output tuple (out [8,2048,64] f32,
gate [8,2048,1] f32).

Sharding: data-parallel over batch (B=8 -> one batch element per NeuronCore).
The 3 embedding tables (+1 zero row) are concatenated into one big row table
replicated to every core.

Per core:
  - rolling n-gram hashes computed ON DEVICE in exact fp32 Barrett arithmetic
    (all intermediates < 2^24; q = rnd(r*(1/p)) via the +1.5*2^23 round trick,
    one conditional correction) -> 24 row indices per token
  - 384 indirect DMA gathers ([128 tokens, 64] rows each; HW semantics allow
    one table-row offset per destination partition per instruction)
  - PE transposes to feature-major, fp32 matmuls for k/v, q from
    host-pre-transposed hidden states, gate = sigmoid(q.k/8) via ones-matmul,
    out = gate*(v@Wo)+bo in natural layout
Self-contained: hardcodes all shapes; imports concourse from /opt/trn_rl_repo.
"""

import os
import sys

import numpy as np

if "/opt/trn_rl_repo" not in sys.path:
    sys.path.insert(0, "/opt/trn_rl_repo")

# ---- problem constants (hardcoded per contract) ----
B, S, D, H = 8, 2048, 64, 8
NGRAMS = [2, 3, 4]
PRIMES = [100003, 100019, 100043]
NCHUNK = S // 128           # 16 chunks of 128 tokens
NBLK = 4                    # process 4 chunks (512 tokens) per pipeline block
CPB = NCHUNK // NBLK        # chunks per block = 4
NCOMBO = len(NGRAMS) * H    # 24 gathers per token
FEAT = NCOMBO * D           # 1536
NF = FEAT // 128            # 12 feature chunks of 128

_OFF = [0]
for _p in PRIMES:
    _OFF.append(_OFF[-1] + H * _p)
R_TOTAL = _OFF[-1]          # 2400520 rows
R_ZERO = R_TOTAL            # appended zero row
N_CORES = 8
RND = 12582912.0            # 1.5 * 2^23: fp32 round-to-nearest-int trick

_CACHED = None


def _build_kernel():
    from contextlib import ExitStack

    import concourse.bass as bass
    import concourse.tile as tile
    from concourse import bacc, mybir
    from concourse.masks import make_identity

    f32 = mybir.dt.float32
    i32 = mybir.dt.int32
    OP = mybir.AluOpType

    nc = bacc.Bacc("TRN2", target_bir_lowering=False, debug=False)

    # ---- DRAM parameters ----
    table = nc.declare_dram_parameter("table", [R_TOTAL + 1, D], f32, isOutput=False)
    tsh_d = nc.declare_dram_parameter("tsh", [128, 4, NCHUNK], f32, isOutput=False)
    hc_d = nc.declare_dram_parameter("hconst", [128, 9, H, NCHUNK], f32,
                                     isOutput=False)
    hb_d = nc.declare_dram_parameter("hbase", [128, 3, H, NCHUNK], f32,
                                     isOutput=False)
    hm_d = nc.declare_dram_parameter("hmask", [128, 3, H, NCHUNK], f32,
                                     isOutput=False)
    hsT_d = nc.declare_dram_parameter("hsT", [D, S], f32, isOutput=False)
    wk_d = nc.declare_dram_parameter("wk", [128, NF, D], f32, isOutput=False)
    wv_d = nc.declare_dram_parameter("wv", [128, NF, D], f32, isOutput=False)
    wq_d = nc.declare_dram_parameter("wq", [D, D], f32, isOutput=False)
    wo_d = nc.declare_dram_parameter("wo", [D, D], f32, isOutput=False)
    bq_d = nc.declare_dram_parameter("bq", [D, 1], f32, isOutput=False)
    bk_d = nc.declare_dram_parameter("bk", [D, 1], f32, isOutput=False)
    bv_d = nc.declare_dram_parameter("bv", [D, 1], f32, isOutput=False)
    bob_d = nc.declare_dram_parameter("bob", [128, CPB, D], f32, isOutput=False)
    out_d = nc.declare_dram_parameter("out_nat", [128, NCHUNK, D], f32, isOutput=True)
    gate_d = nc.declare_dram_parameter("gate_nat", [128, NCHUNK], f32, isOutput=True)

    with tile.TileContext(nc) as tc, ExitStack() as ctx:
        const_p = ctx.enter_context(tc.tile_pool(name="const", bufs=1))
        hash_p = ctx.enter_context(tc.tile_pool(name="hash", bufs=8))
        mem_p = ctx.enter_context(tc.tile_pool(name="mem", bufs=2))
        memT_p = ctx.enter_context(tc.tile_pool(name="memT", bufs=2))
        small_p = ctx.enter_context(tc.tile_pool(name="small", bufs=2))
        ps_t = ctx.enter_context(tc.tile_pool(name="ps_t", bufs=3, space="PSUM"))
        ps_mm = ctx.enter_context(tc.tile_pool(name="ps_mm", bufs=2, space="PSUM"))
        ps_g = ctx.enter_context(tc.tile_pool(name="ps_g", bufs=1, space="PSUM"))
        ps_vn = ctx.enter_context(tc.tile_pool(name="ps_vn", bufs=2, space="PSUM"))

        # ---- constants / weights ----
        id128 = const_p.tile([128, 128], f32)
        make_identity(nc, id128[:])
        id64 = const_p.tile([64, 64], f32)
        make_identity(nc, id64[:])
        ones64 = const_p.tile([D, 1], f32)
        nc.gpsimd.memset(ones64[:], 1.0)

        wk_sb = const_p.tile([128, NF, D], f32)
        nc.sync.dma_start(wk_sb[:], wk_d[:])
        wv_sb = const_p.tile([128, NF, D], f32)
        nc.sync.dma_start(wv_sb[:], wv_d[:])
        wq_sb = const_p.tile([D, D], f32)
        nc.sync.dma_start(wq_sb[:], wq_d[:])
        wo_sb = const_p.tile([D, D], f32)
        nc.sync.dma_start(wo_sb[:], wo_d[:])
        bq_sb = const_p.tile([D, 1], f32)
        nc.sync.dma_start(bq_sb[:], bq_d[:])
        bk_sb = const_p.tile([D, 1], f32)
        nc.sync.dma_start(bk_sb[:], bk_d[:])
        bv_sb = const_p.tile([D, 1], f32)
        nc.sync.dma_start(bv_sb[:], bv_d[:])
        bob_sb = const_p.tile([128, CPB, D], f32)
        nc.sync.dma_start(bob_sb[:], bob_d[:])
        tsh_sb = const_p.tile([128, 4, NCHUNK], f32)
        nc.sync.dma_start(tsh_sb[:], tsh_d[:])
        hc_sb = const_p.tile([128, 9, H, NCHUNK], f32)
        nc.sync.dma_start(hc_sb[:], hc_d[:])
        hb_sb = const_p.tile([128, 3, H, NCHUNK], f32)
        nc.sync.dma_start(hb_sb[:], hb_d[:])
        hm_sb = const_p.tile([128, 3, H, NCHUNK], f32)
        nc.sync.dma_start(hm_sb[:], hm_d[:])
        hsT_sb = const_p.tile([D, S], f32)
        nc.sync.dma_start(hsT_sb[:], hsT_d[:])

        # combo-major index tile: idx[p, combo, chunk]
        idx_sb = const_p.tile([128, NCOMBO, NCHUNK], i32)
        out_all = const_p.tile([128, NCHUNK, D], f32)
        gate_all = const_p.tile([128, NCHUNK], f32)

        SH = [128, H, NCHUNK]  # hash working shape

        def ht():
            return hash_p.tile(SH, f32, tag="h", name="htmp")

        def mod_p(r, p, invp):
            """r mod p for exact-int f32 r < 2^24, quotient < ~200."""
            qf = ht()
            nc.vector.tensor_scalar(qf[:], r, float(invp), None, OP.mult)
            qi = ht()
            nc.vector.tensor_scalar(qi[:], qf[:], RND, -RND, OP.add, OP.add)
            qp = ht()
            nc.vector.tensor_scalar(qp[:], qi[:], float(p), None, OP.mult)
            r2 = ht()
            nc.vector.tensor_tensor(r2[:], r, qp[:], OP.subtract)
            co = ht()
            nc.vector.tensor_scalar(co[:], r2[:], 0.0, float(p), OP.is_lt, OP.mult)
            r3 = ht()
            nc.vector.tensor_tensor(r3[:], r2[:], co[:], OP.add)
            return r3[:]

        # ---- device hash: h = (h*m + t) mod p, m = c2*4096 + c1*64 + c0 ----
        for g, (n, p) in enumerate(zip(NGRAMS, PRIMES)):
            invp = float(np.float32(1.0) / np.float32(p))
            c2 = hc_sb[:, g * 3 + 0, :, :]
            c1 = hc_sb[:, g * 3 + 1, :, :]
            c0 = hc_sb[:, g * 3 + 2, :, :]
            h = tsh_sb[:, n - 1:n, :].to_broadcast(SH)  # h_1 = t_{n-1} < p
            for s in range(n - 2, -1, -1):
                tb = tsh_sb[:, s:s + 1, :].to_broadcast(SH)
                u2 = ht()
                nc.vector.tensor_tensor(u2[:], h, c2, OP.mult)
                m1 = mod_p(u2[:], p, invp)
                u3 = ht()
                nc.vector.tensor_scalar(u3[:], m1, 64.0, None, OP.mult)
                t1 = ht()
                nc.vector.tensor_tensor(t1[:], h, c1, OP.mult)
                u3b = ht()
                nc.vector.tensor_tensor(u3b[:], u3[:], t1[:], OP.add)
                m2 = mod_p(u3b[:], p, invp)
                u4 = ht()
                nc.vector.tensor_scalar(u4[:], m2, 64.0, None, OP.mult)
                t2 = ht()
                nc.vector.tensor_tensor(t2[:], h, c0, OP.mult)
                u4b = ht()
                nc.vector.tensor_tensor(u4b[:], u4[:], t2[:], OP.add)
                u4c = ht()
                nc.vector.tensor_tensor(u4c[:], u4b[:], tb, OP.add)
                h = mod_p(u4c[:], p, invp)
            # mask (pos < n-1): idx = h*mask + (base | R_ZERO)
            hmk = ht()
            nc.vector.tensor_tensor(hmk[:], h, hm_sb[:, g, :, :], OP.mult)
            idxf = ht()
            nc.vector.tensor_tensor(idxf[:], hmk[:], hb_sb[:, g, :, :], OP.add)
            # cast to int32 into combo-major idx (values integral: rint ok)
            nc.vector.tensor_copy(idx_sb[:, g * H:(g + 1) * H, :], idxf[:])

        for blk in range(NBLK):
            tok0 = blk * 512
            # ---- gather: one [128,64] indirect DMA per (chunk, combo) ----
            mem = mem_p.tile([128, CPB, FEAT], f32, tag="mem")
            for c4 in range(CPB):
                c = blk * CPB + c4
                for k in range(NCOMBO):
                    nc.gpsimd.indirect_dma_start(
                        out=mem[:, c4, k * D:(k + 1) * D],
                        out_offset=None,
                        in_=table[:],
                        in_offset=bass.IndirectOffsetOnAxis(
                            ap=idx_sb[:, k, c:c + 1], axis=0),
                    )
            # ---- transpose to feature-major [128, NF, 512] ----
            memT = memT_p.tile([128, NF, 512], f32, tag="memT")
            flip = 0
            for c4 in range(CPB):
                for f in range(NF):
                    pt = ps_t.tile([128, 128], f32, tag="pt")
                    nc.tensor.transpose(
                        pt[:], mem[:, c4, f * 128:(f + 1) * 128], id128[:]
                    )
                    dst = memT[:, f, c4 * 128:(c4 + 1) * 128]
                    if flip % 2 == 0:
                        nc.scalar.copy(dst, pt[:])
                    else:
                        nc.vector.tensor_copy(dst, pt[:])
                    flip += 1

            # ---- q^T = Wq^T @ hs^T + bq ----
            qp = ps_mm.tile([D, 512], f32, tag="mm")
            nc.tensor.matmul(qp[:], wq_sb[:], hsT_sb[:, tok0:tok0 + 512],
                             start=True, stop=True)
            q_sb = small_p.tile([D, 512], f32, tag="q")
            nc.vector.tensor_scalar(q_sb[:], qp[:], bq_sb[:, 0:1], None, OP.add)

            # ---- k^T = Wk^T @ mem^T + bk ----
            kp = ps_mm.tile([D, 512], f32, tag="mm")
            for f in range(NF):
                nc.tensor.matmul(kp[:], wk_sb[:, f, :], memT[:, f, :],
                                 start=(f == 0), stop=(f == NF - 1))
            k_sb = small_p.tile([D, 512], f32, tag="k")
            nc.vector.tensor_scalar(k_sb[:], kp[:], bk_sb[:, 0:1], None, OP.add)

            # ---- gate = sigmoid(sum_f q*k / 8), natural layout ----
            prod = small_p.tile([D, 512], f32, tag="prod")
            nc.vector.tensor_tensor(prod[:], q_sb[:], k_sb[:], OP.mult)
            gp = ps_g.tile([128, CPB], f32, tag="g")
            for c4 in range(CPB):
                nc.tensor.matmul(gp[:, c4:c4 + 1],
                                 prod[:, c4 * 128:(c4 + 1) * 128],
                                 ones64[:], start=True, stop=True)
            nc.scalar.activation(gate_all[:, blk * CPB:(blk + 1) * CPB], gp[:],
                                 mybir.ActivationFunctionType.Sigmoid,
                                 scale=0.125)

            # ---- v^T then (v @ Wo)^T ----
            vp = ps_mm.tile([D, 512], f32, tag="mm")
            for f in range(NF):
                nc.tensor.matmul(vp[:], wv_sb[:, f, :], memT[:, f, :],
                                 start=(f == 0), stop=(f == NF - 1))
            v_sb = small_p.tile([D, 512], f32, tag="v")
            nc.vector.tensor_scalar(v_sb[:], vp[:], bv_sb[:, 0:1], None, OP.add)
            vwp = ps_mm.tile([D, 512], f32, tag="mm")
            nc.tensor.matmul(vwp[:], wo_sb[:], v_sb[:], start=True, stop=True)
            vw_sb = small_p.tile([D, 512], f32, tag="vw")
            nc.scalar.copy(vw_sb[:], vwp[:])

            # ---- out = gate * (v@Wo) + bo, natural layout ----
            for c4 in range(CPB):
                c = blk * CPB + c4
                vn = ps_vn.tile([128, D], f32, tag="vn")
                nc.tensor.transpose(vn[:], vw_sb[:, c4 * 128:(c4 + 1) * 128],
                                    id64[:])
                nc.vector.tensor_scalar(out_all[:, c, :], vn[:],
                                        gate_all[:, c:c + 1], None, OP.mult)
            nc.vector.tensor_tensor(
                out_all[:, blk * CPB:(blk + 1) * CPB, :],
                out_all[:, blk * CPB:(blk + 1) * CPB, :],
                bob_sb[:], OP.add)

        nc.sync.dma_start(out_d[:], out_all[:])
        nc.sync.dma_start(gate_d[:], gate_all[:])

    nc.compile()
    return nc


def _get_kernel():
    global _CACHED
    if _CACHED is None:
        _CACHED = _build_kernel()
    return _CACHED


def _prep_in_maps(inputs: dict) -> list:
    """Host-side shard/layout prep: returns list of 8 per-core input dicts."""
    token_ids = np.asarray(inputs["token_ids"]).astype(np.int64)
    hs = np.asarray(inputs["hidden_states"], dtype=np.float32)
    tables = [np.asarray(inputs[f"table_n{n}"], dtype=np.float32) for n in NGRAMS]
    mults = [np.asarray(inputs[f"mult_n{n}"]).astype(np.int64) for n in NGRAMS]
    Wq = np.asarray(inputs["Wq"], dtype=np.float32)
    Wk = np.asarray(inputs["Wk"], dtype=np.float32)
    Wv = np.asarray(inputs["Wv"], dtype=np.float32)
    Wo = np.asarray(inputs["Wo"], dtype=np.float32)
    bq = np.asarray(inputs["bq"], dtype=np.float32)
    bk = np.asarray(inputs["bk"], dtype=np.float32)
    bv = np.asarray(inputs["bv"], dtype=np.float32)
    bo = np.asarray(inputs["bo"], dtype=np.float32)

    big = np.empty((R_TOTAL + 1, D), dtype=np.float32)
    for g in range(3):
        big[_OFF[g]:_OFF[g + 1]] = tables[g].reshape(H * PRIMES[g], D)
    big[R_ZERO] = 0.0

    # hash constants: m = c2*4096 + c1*64 + c0 per (g, head), replicated
    hconst = np.empty((128, 9, H, NCHUNK), dtype=np.float32)
    hbase = np.empty((128, 3, H, NCHUNK), dtype=np.float32)
    hmask = np.ones((128, 3, H, NCHUNK), dtype=np.float32)
    for g in range(3):
        m = mults[g]
        n = NGRAMS[g]
        for piece, val in enumerate([m >> 12, (m >> 6) & 63, m & 63]):
            hconst[:, g * 3 + piece, :, :] = \
                val.astype(np.float32)[None, :, None]
        base = _OFF[g] + np.arange(H, dtype=np.int64) * PRIMES[g]
        hbase[:, g, :, :] = base.astype(np.float32)[None, :, None]
        # positions pos = c*128 + p < n-1 are masked to the zero row
        hmask[0:n - 1, g, :, 0] = 0.0
        hbase[0:n - 1, g, :, 0] = float(R_ZERO)

    wk_sb = np.ascontiguousarray(Wk.reshape(NF, 128, D).transpose(1, 0, 2))
    wv_sb = np.ascontiguousarray(Wv.reshape(NF, 128, D).transpose(1, 0, 2))
    bob = np.broadcast_to(bo[None, None, :], (128, CPB, D)).copy()

    common = {
        "table": big, "hconst": hconst, "hbase": hbase, "hmask": hmask,
        "wk": wk_sb, "wv": wv_sb, "wq": Wq, "wo": Wo,
        "bq": bq.reshape(D, 1), "bk": bk.reshape(D, 1),
        "bv": bv.reshape(D, 1), "bob": bob,
    }
    in_maps = []
    for b in range(B):
        tok = token_ids[b]
        # tsh[p, s, c] = token[c*128 + p - s] (0 when out of range), f32-exact
        tsh = np.zeros((128, 4, NCHUNK), dtype=np.float32)
        for s_ in range(4):
            shifted = np.zeros(S, dtype=np.int64)
            if s_ == 0:
                shifted[:] = tok
            else:
                shifted[s_:] = tok[:-s_]
            tsh[:, s_, :] = shifted.reshape(NCHUNK, 128).T
        m = dict(common)
        m["tsh"] = tsh
        m["hsT"] = np.ascontiguousarray(hs[b].T)
        in_maps.append(m)
    return in_maps


def _unshard(results: list) -> tuple:
    out = np.empty((B, S, D), dtype=np.float32)
    gate = np.empty((B, S, 1), dtype=np.float32)
    for b in range(B):
        o = results[b]["out_nat"]          # [128, 16, 64]
        g = results[b]["gate_nat"]         # [128, 16]
        out[b] = o.transpose(1, 0, 2).reshape(S, D)
        gate[b] = g.T.reshape(S, 1)
    return out, gate


def kernel(**inputs):
    from concourse.bass_utils import run_bass_kernel_spmd

    nc = _get_kernel()
    in_maps = _prep_in_maps(inputs)
    trace = bool(os.environ.get("ENGRAM_TRACE"))
    if trace:
        sys.path.insert(0, "/root/problem")
        try:
            import ntff_shim
            ntff_shim.install()
        except Exception:
            pass
    res = run_bass_kernel_spmd(nc, in_maps, list(range(N_CORES)), trace=trace)
    if trace:
        kernel.last_exec_time_ns = res.exec_time_ns
        kernel.last_results = res
    return _unshard(res.results)


kernel.last_exec_time_ns = None
kernel.last_results = None
